# revision 1
# baseline (speedup 1.0000x reference)
"""GCMC layer on trn2 — dma_gather + one-hot PE segment-sum implementation.

Design (per core, dst-sharded: core c owns dst nodes [c*NSH, (c+1)*NSH)):
  - fsrc table [2*NU, 64] in DRAM = feat * cj for both sides (built on device).
  - Per (dir, rating) stream: edges with dst in shard, grouped host-side by
    (src-slab, dst-tile). Slabs are 32768-row windows of the table so gather
    indices fit int16 (dma_gather requirement). Each (slab, tile) segment is
    padded to a multiple of 128 edges (chunk). Segment sizes are maxed across
    cores so the program is SPMD-uniform; padding slots gather garbage rows
    but carry dstloc = -1 which the one-hot kills.
  - dma_gather (SWDGE CounterMachine path, ~9ns/row vs ~1.6us/row for generic
    indirect DMA) pulls 256B rows into xb tiles, edge j at [j%128, j//128].
  - Segment-sum on the PE: per 128-edge chunk, one-hot O[e, d] =
    (dstloc[e] == iota[d]) built by one DVE is_equal per 16 chunks;
    psum[d, :] += O^T @ X accumulates a dst-tile across its chunks, then a
    DVE add flushes into the SBUF-resident Y_acc [128, R*TP*64].
  - Transform per (dir, tile): per-r PE transpose + matmul with W_r
    (basis-combined), relu(msg * ci) on ACT, PE transpose + fc matmul with
    bias, dense store. No indirect DMA anywhere except the gathers.
Host only bins/sorts indices and packs int16 index blocks.
"""
import sys
import numpy as np

sys.path.insert(0, '/opt/trn_rl_repo')

import ml_dtypes
BF = ml_dtypes.bfloat16

import concourse.bass as bass
import concourse.tile as tile
import concourse.mybir as mybir
from concourse import bacc
from concourse.masks import make_identity

F32 = mybir.dt.float32
BF16 = mybir.dt.bfloat16
I16 = mybir.dt.int16
ALU = mybir.AluOpType
ACTF = mybir.ActivationFunctionType
P = 128


class Cfg:
    def __init__(self, NU=100000, NM=100000, R=5, E=1000000, IN=64, BAS=4,
                 NCORES=8):
        assert NU == NM
        self.NU, self.NM, self.R, self.E, self.IN, self.BAS = NU, NM, R, E, IN, BAS
        self.MPR = 16
        self.MSG = self.MPR * R
        self.OUT = 64
        self.NCORES = NCORES
        self.NSH = NU // NCORES               # dsts per core
        self.TP = -(-self.NSH // P)           # dst tiles per direction
        self.SLAB = 32768
        self.NSLAB = -(-NU // self.SLAB)
        self.CALL_CH = 64                     # chunks (128 rows) per gather call
        self.OHC = 12                         # chunks per one-hot DVE block


# ----------------------------------------------------------------- host prep

def build_plan(cfg, edge_user, edge_movie):
    """Group edges per (dir, r, core) by (src-slab, dst-tile); pad each
    segment to the max chunk count across cores (SPMD-uniform structure).

    Returns:
      streams: per (d, r): dict(calls=[dict(slab, segs=[(t, ch)], rows,
               c16ofs, ccofs)], rows)
      gidx:  per-core int16 [128, G16] gather-index blocks
      dloc:  per-core f32  [128, GCC] in-tile dst ids (-1 = padding)
    """
    NC, NSH, TP, R = cfg.NCORES, cfg.NSH, cfg.TP, cfg.R
    NSLAB, SLAB = cfg.NSLAB, cfg.SLAB
    eu = np.asarray(edge_user)
    em = np.asarray(edge_movie)

    streams = []
    gblocks = [[] for _ in range(NC)]
    dblocks = [[] for _ in range(NC)]
    c16ofs = 0
    ccofs = 0
    for d in range(2):
        src_all, dst_all = (eu, em) if d == 0 else (em, eu)
        side = 0 if d == 0 else 1
        for r in range(R):
            src, dst = src_all[r], dst_all[r]
            core = dst // NSH
            pc = []
            cnt = np.zeros((NC, NSLAB, TP), np.int64)
            for c in range(NC):
                m = core == c
                s = src[m].astype(np.int64)
                ld = (dst[m] - c * NSH).astype(np.int64)
                slab = s >> 15
                t = ld >> 7
                np.add.at(cnt[c], (slab, t), 1)
                pc.append((s, ld, slab, t))
            ch = -(-(cnt.max(axis=0)) // P)          # [NSLAB, TP] chunks

            # greedy call packing over (slab, tile) segments
            calls = []
            cur = None
            for sb in range(NSLAB):
                for t in range(TP):
                    n = int(ch[sb, t])
                    if n == 0:
                        continue
                    if cur is None or cur['slab'] != sb or \
                       cur['ch'] + n > cfg.CALL_CH:
                        if cur is not None:
                            calls.append(cur)
                        cur = dict(slab=sb, segs=[], ch=0)
                    cur['segs'].append((t, n))
                    cur['ch'] += n
            if cur is not None:
                calls.append(cur)

            # per-segment padded row offsets (same for all cores)
            rows_tot = int(ch.sum()) * P
            seg_ofs = {}
            o = 0
            for cl in calls:
                cl['rows'] = cl['ch'] * P
                cl['rowofs'] = o
                for t, n in cl['segs']:
                    seg_ofs[(cl['slab'], t)] = o
                    o += n * P

            # per-core padded streams
            for c in range(NC):
                s, ld, slab, t = pc[c]
                order = np.lexsort((ld, slab))
                s, ld, slab, t = s[order], ld[order], slab[order], t[order]
                # rank within (slab, tile) group
                key = slab * TP + t
                bnd = np.flatnonzero(np.diff(key, prepend=-1))
                rank = np.arange(key.size) - np.repeat(
                    bnd, np.diff(np.append(bnd, key.size)))
                base = np.array([seg_ofs.get((int(sb), int(tt)), -1)
                                 for sb, tt in zip(slab[bnd], t[bnd])])
                pos = np.repeat(base, np.diff(np.append(bnd, key.size))) + rank
                gs = np.zeros(rows_tot, np.int16)
                dl = np.full(rows_tot, -1.0, np.float32)
                gs[pos] = (s & 32767).astype(np.int16)
                dl[pos] = (ld - (t << 7)).astype(np.float32)
                gb = np.tile(gs.reshape(-1, 16).T, (8, 1))      # [128, rows/16]
                db = np.ascontiguousarray(
                    dl.reshape(-1, P).T)                         # [128, rows/128]
                gblocks[c].append(gb)
                dblocks[c].append(db)

            for cl in calls:
                cl['c16ofs'] = c16ofs + cl['rowofs'] // 16
                cl['ccofs'] = ccofs + cl['rowofs'] // P
            streams.append(dict(d=d, r=r, side=side, calls=calls,
                                rows=rows_tot))
            c16ofs += rows_tot // 16
            ccofs += rows_tot // P

    gidx = [np.concatenate(g, axis=1) for g in gblocks]
    dloc = [np.concatenate(db, axis=1) for db in dblocks]
    return streams, gidx, dloc


# ------------------------------------------------------------- numpy model

def model(cfg, streams, gidx, dloc, inputs):
    """Numpy mirror of the device program, for validation."""
    R, NSH, TP, IN = cfg.R, cfg.NSH, cfg.TP, cfg.IN
    W = np.einsum('rb,bio->rio', np.asarray(inputs['att']),
                  np.asarray(inputs['basis'])).astype(np.float32)
    fc_w = np.asarray(inputs['fc_w'])
    fc_b = np.asarray(inputs['fc_b'])
    tbl = np.concatenate([
        np.asarray(inputs['ufeat']) * np.asarray(inputs['cj_user']),
        np.asarray(inputs['ifeat']) * np.asarray(inputs['cj_movie'])],
        axis=0).astype(np.float32)
    ci = [np.asarray(inputs['ci_movie']), np.asarray(inputs['ci_user'])]

    u_out = np.zeros((cfg.NU, cfg.OUT), np.float32)
    m_out = np.zeros((cfg.NM, cfg.OUT), np.float32)
    for c in range(cfg.NCORES):
        for d in range(2):
            yacc = np.zeros((R, TP * P, IN), np.float32)
            for st in streams:
                if st['d'] != d:
                    continue
                r = st['r']
                for cl in st['calls']:
                    base = st['side'] * cfg.NU + cl['slab'] * cfg.SLAB
                    nrow = min(cfg.SLAB, cfg.NU - cl['slab'] * cfg.SLAB)
                    o = cl['rowofs']
                    gs = gidx[c][:, cl['c16ofs']:cl['c16ofs'] + cl['rows'] // 16]
                    dl = dloc[c][:, cl['ccofs']:cl['ccofs'] + cl['rows'] // P]
                    lin_g = gs[:16].T.reshape(-1)
                    lin_d = dl.T.reshape(-1)
                    x = tbl[base + lin_g.astype(np.int64)]
                    co = 0
                    for t, n in cl['segs']:
                        seg_d = lin_d[co:co + n * P]
                        seg_x = x[co:co + n * P]
                        msk = seg_d >= 0
                        np.add.at(yacc[r], t * P + seg_d[msk].astype(np.int64),
                                  seg_x[msk])
                        co += n * P
            msgs = np.zeros((TP * P, cfg.MSG), np.float32)
            for t in range(TP):
                for r in range(R):
                    msgs[t * P:(t + 1) * P, r * 16:(r + 1) * 16] = \
                        yacc[r, t * P:(t + 1) * P] @ W[r]
            cish = np.zeros((TP * P, 1), np.float32)
            cish[:NSH] = ci[d][c * NSH:(c + 1) * NSH]
            z = np.maximum(msgs * cish, 0.0) @ fc_w.T + fc_b
            if d == 0:
                m_out[c * NSH:(c + 1) * NSH] = z[:NSH]
            else:
                u_out[c * NSH:(c + 1) * NSH] = z[:NSH]
    return u_out, m_out


# ---------------------------------------------------------- device program

def build_program(cfg, streams, g16cols, gccols, num_devices):
    nc = bacc.Bacc("TRN2", target_bir_lowering=False, debug=False,
                   num_devices=num_devices)
    NU, IN, R, BAS = cfg.NU, cfg.IN, cfg.R, cfg.BAS
    TP = cfg.TP

    ufeat = nc.dram_tensor("ufeat", (NU, IN), F32, kind="ExternalInput")
    ifeat = nc.dram_tensor("ifeat", (NU, IN), F32, kind="ExternalInput")
    cj_u = nc.dram_tensor("cj_u", (NU, 1), F32, kind="ExternalInput")
    cj_m = nc.dram_tensor("cj_m", (NU, 1), F32, kind="ExternalInput")
    ci_sh = nc.dram_tensor("ci_sh", (2 * TP * P, 1), F32, kind="ExternalInput")
    attT = nc.dram_tensor("attT", (BAS, R), F32, kind="ExternalInput")
    basis2 = nc.dram_tensor("basis2", (BAS, IN * 16), F32, kind="ExternalInput")
    fc_w = nc.dram_tensor("fc_w", (64, cfg.MSG), F32, kind="ExternalInput")
    fc_b = nc.dram_tensor("fc_b", (1, 64), F32, kind="ExternalInput")
    iota_d = nc.dram_tensor("iota128", (P, P), F32, kind="ExternalInput")
    gidx = nc.dram_tensor("gidx", (P, g16cols), I16, kind="ExternalInput")
    dloc = nc.dram_tensor("dloc", (P, gccols), F32, kind="ExternalInput")

    m_out = nc.dram_tensor("m_out", (TP * P, 64), F32, kind="ExternalOutput")
    u_out = nc.dram_tensor("u_out", (TP * P, 64), F32, kind="ExternalOutput")

    fsrc = nc.dram_tensor("fsrc", (2 * NU, IN), F32, kind="Internal")
    wscr = nc.dram_tensor("wscr", (R, IN * 16), F32, kind="Internal")

    with tile.TileContext(nc) as tc:
        with tc.tile_pool(name="const", bufs=1) as pool:
            # ---------------- constants ----------------
            pp_ctx = tc.tile_pool(name="cpsum", bufs=2, space="PSUM")
            pp = pp_ctx.__enter__()
            ident = pool.tile([P, P], F32)
            make_identity(nc, ident[:])
            ident16 = pool.tile([P, P], BF16)
            make_identity(nc, ident16[:])

            with tc.tile_pool(name="w0", bufs=1) as wp:
                at = wp.tile([BAS, R], F32)
                bs = wp.tile([BAS, IN * 16], F32)
                nc.sync.dma_start(out=at[:], in_=attT.ap()[:])
                nc.sync.dma_start(out=bs[:], in_=basis2.ap()[:])
                w5 = wp.tile([R, IN * 16], F32)
                half = IN * 16 // 2
                for h in range(2):
                    ps = pp.tile([R, half], F32, space="PSUM", tag="w5ps")
                    nc.tensor.matmul(out=ps[:], lhsT=at[:],
                                     rhs=bs[:, h * half:(h + 1) * half],
                                     start=True, stop=True)
                    nc.scalar.copy(out=w5[:, h * half:(h + 1) * half], in_=ps[:])
                nc.sync.dma_start(out=wscr.ap()[:], in_=w5[:])
            w64 = pool.tile([IN, R, 16], F32)
            nc.sync.dma_start(
                out=w64[:], in_=wscr.ap()[:].rearrange("r (k o) -> k r o", k=IN))
            w64b = pool.tile([IN, R, 16], BF16)
            nc.scalar.copy(out=w64b[:], in_=w64[:])

            fcw = pool.tile([64, cfg.MSG], F32)
            nc.sync.dma_start(out=fcw[:], in_=fc_w.ap()[:])
            psT = pp.tile([cfg.MSG, 64], F32, space="PSUM", tag="fcT")
            nc.tensor.transpose(out=psT[:], in_=fcw[:], identity=ident[:64, :64])
            fcwT = pool.tile([cfg.MSG, 64], F32)
            nc.scalar.copy(out=fcwT[:], in_=psT[:])
            fcb = pool.tile([P, 64], F32)
            nc.sync.dma_start(out=fcb[:], in_=fc_b.ap()[:].to_broadcast((P, 64)))

            cisb = pool.tile([P, 2 * TP], F32)
            nc.sync.dma_start(
                out=cisb[:],
                in_=ci_sh.ap()[:].rearrange("(t p) o -> p (t o)", p=P))
            iota = pool.tile([P, 1, P], F32)
            nc.sync.dma_start(out=iota[:, 0, :], in_=iota_d.ap()[:])
            pp_ctx.__exit__(None, None, None)

            # ---------------- table build ----------------
            with tc.tile_pool(name="p1", bufs=3) as p1:
                GT = 8
                for side, (feat, cj) in enumerate(((ufeat, cj_u), (ifeat, cj_m))):
                    starts = list(range(0, NU - GT * P + 1, GT * P))
                    if NU % (GT * P):
                        starts.append(NU - GT * P)
                    for g0 in starts:
                        ft = p1.tile([P, GT, IN], F32, tag="ft")
                        cjt = p1.tile([P, GT, 1], F32, tag="cj")
                        nc.sync.dma_start(
                            out=ft[:], in_=feat.ap()[g0:g0 + GT * P].rearrange(
                                "(p a) d -> p a d", p=P))
                        nc.sync.dma_start(
                            out=cjt[:], in_=cj.ap()[g0:g0 + GT * P].rearrange(
                                "(p a) d -> p a d", p=P))
                        sc = p1.tile([P, GT, IN], F32, tag="sc")
                        nc.vector.tensor_tensor(
                            out=sc[:], in0=ft[:],
                            in1=cjt[:].to_broadcast((P, GT, IN)),
                            op=ALU.mult)
                        nc.sync.dma_start(
                            out=fsrc.ap()[side * NU + g0: side * NU + g0 + GT * P]
                                .rearrange("(p a) d -> p a d", p=P),
                            in_=sc[:])

            # ---------------- gather + segment-sum + transform ----------------
            CC = cfg.CALL_CH
            with tc.tile_pool(name="yacc", bufs=2) as yp, \
                 tc.tile_pool(name="io", bufs=3) as iop, \
                 tc.tile_pool(name="xb", bufs=3) as xp, \
                 tc.tile_pool(name="xc", bufs=2) as xcp, \
                 tc.tile_pool(name="oh", bufs=2) as ohp, \
                 tc.tile_pool(name="ps", bufs=3, space="PSUM") as psp, \
                 tc.tile_pool(name="p3", bufs=3) as p3, \
                 tc.tile_pool(name="p3ps", bufs=1, space="PSUM") as p3p:
                for d in range(2):
                    yacc = yp.tile([P, R * TP * IN], BF16, tag="yacc")
                    nc.vector.memset(yacc[:], 0.0)
                    for st in streams:
                        if st['d'] != d:
                            continue
                        r = st['r']
                        for cl in st['calls']:
                            rows = cl['rows']
                            cc = rows // P
                            c16 = rows // 16
                            base = st['side'] * NU + cl['slab'] * cfg.SLAB
                            nrow = min(cfg.SLAB, NU - cl['slab'] * cfg.SLAB)
                            gt = iop.tile([P, CC * 8], I16, tag="gt")
                            nc.sync.dma_start(
                                out=gt[:, :c16],
                                in_=gidx.ap()[:, cl['c16ofs']:cl['c16ofs'] + c16])
                            dl = iop.tile([P, CC, 1], F32, tag="dl")
                            nc.sync.dma_start(
                                out=dl[:, :cc, :],
                                in_=dloc.ap()[:, cl['ccofs']:cl['ccofs'] + cc]
                                    .rearrange("p (c one) -> p c one", one=1))
                            xb = xp.tile([P, CC, IN], F32, tag="xb")
                            nc.gpsimd.dma_gather(
                                xb[:, :cc, :], fsrc.ap()[base:base + nrow],
                                gt[:, :c16], rows, rows, IN,
                                single_packet=False)
                            xb16 = xcp.tile([P, CC, IN], BF16, tag="xb16")
                            nc.scalar.copy(out=xb16[:, :cc, :],
                                           in_=xb[:, :cc, :])
                            # chunk -> (tile, first, last)
                            chmeta = []
                            for t, n in cl['segs']:
                                for k in range(n):
                                    chmeta.append((t, k == 0, k == n - 1))
                            ps = None
                            for ob in range(0, cc, cfg.OHC):
                                obc = min(cfg.OHC, cc - ob)
                                oh = ohp.tile([P, cfg.OHC, P], BF16, tag="oh")
                                nc.vector.tensor_tensor(
                                    out=oh[:, :obc, :],
                                    in0=dl[:, ob:ob + obc, :].to_broadcast(
                                        (P, obc, P)),
                                    in1=iota[:, 0:1, :].to_broadcast(
                                        (P, obc, P)),
                                    op=ALU.is_equal)
                                for j in range(obc):
                                    t, first, last = chmeta[ob + j]
                                    if first:
                                        ps = psp.tile([P, IN], F32,
                                                      space="PSUM", tag="ps")
                                    nc.tensor.matmul(
                                        out=ps[:], lhsT=oh[:, j, :],
                                        rhs=xb16[:, ob + j, :],
                                        start=first, stop=last)
                                    if last:
                                        ys = yacc[:, (r * TP + t) * IN:
                                                  (r * TP + t + 1) * IN]
                                        nc.vector.tensor_tensor(
                                            out=ys, in0=ys, in1=ps[:],
                                            op=ALU.add)
                    # ---------------- transform ----------------
                    for t in range(TP):
                        msgp = p3p.tile([P, cfg.MSG], F32, space="PSUM",
                                        tag="msgp")
                        for r in range(R):
                            ys = yacc[:, (r * TP + t) * IN:(r * TP + t + 1) * IN]
                            pst = p3p.tile([IN, P], BF16, space="PSUM", tag="pst")
                            nc.tensor.transpose(out=pst[:], in_=ys,
                                                identity=ident16[:])
                            yT = p3.tile([IN, P], BF16, tag="yT")
                            if r % 2 == 0:
                                nc.scalar.copy(out=yT[:], in_=pst[:])
                            else:
                                nc.vector.tensor_copy(out=yT[:], in_=pst[:])
                            nc.tensor.matmul(
                                out=msgp[:, r * 16:(r + 1) * 16],
                                lhsT=yT[:], rhs=w64b[:, r, :],
                                start=True, stop=True)
                        msg = p3.tile([P, cfg.MSG], F32, tag="msg")
                        nc.scalar.activation(
                            out=msg[:], in_=msgp[:],
                            func=ACTF.Relu,
                            scale=cisb[:, d * TP + t: d * TP + t + 1])
                        psmT = p3p.tile([cfg.MSG, P], F32, space="PSUM",
                                        tag="psmT")
                        nc.tensor.transpose(out=psmT[:], in_=msg[:],
                                            identity=ident[:])
                        msgT = p3.tile([cfg.MSG, P], F32, tag="msgT")
                        nc.scalar.copy(out=msgT[:], in_=psmT[:])
                        fcp = p3p.tile([P, 64], F32, space="PSUM", tag="fcp")
                        nc.tensor.matmul(
                            out=fcp[:], lhsT=msgT[:], rhs=fcwT[:],
                            start=True, stop=True)
                        osb = p3.tile([P, 64], F32, tag="osb")
                        nc.vector.tensor_tensor(out=osb[:], in0=fcp[:],
                                                in1=fcb[:], op=ALU.add)
                        dst = m_out if d == 0 else u_out
                        nc.sync.dma_start(
                            out=dst.ap()[t * P:(t + 1) * P], in_=osb[:])
    nc.compile()
    return nc


# ----------------------------------------------------------------- kernel

def make_in_maps(cfg, gidx, dloc, inputs):
    ins = {k: np.asarray(v) for k, v in inputs.items()}
    iota = np.tile(np.arange(P, dtype=np.float32), (P, 1))
    base = dict(
        ufeat=ins['ufeat'], ifeat=ins['ifeat'],
        cj_u=ins['cj_user'], cj_m=ins['cj_movie'],
        attT=np.ascontiguousarray(ins['att'].T),
        basis2=ins['basis'].reshape(cfg.BAS, cfg.IN * 16).copy(),
        fc_w=ins['fc_w'], fc_b=ins['fc_b'].reshape(1, 64).copy(),
        iota128=iota,
    )
    in_maps = []
    for c in range(cfg.NCORES):
        ci = np.zeros((2 * cfg.TP * P, 1), np.float32)
        ci[:cfg.NSH] = ins['ci_movie'][c * cfg.NSH:(c + 1) * cfg.NSH]
        ci[cfg.TP * P:cfg.TP * P + cfg.NSH] = \
            ins['ci_user'][c * cfg.NSH:(c + 1) * cfg.NSH]
        in_maps.append({**base, 'ci_sh': ci, 'gidx': gidx[c], 'dloc': dloc[c]})
    return in_maps


def assemble(cfg, results):
    u = np.concatenate([results[c]['u_out'][:cfg.NSH]
                        for c in range(cfg.NCORES)])
    m = np.concatenate([results[c]['m_out'][:cfg.NSH]
                        for c in range(cfg.NCORES)])
    return u, m


def kernel(**inputs):
    from concourse import bass_utils
    cfg = Cfg()
    streams, gidx, dloc = build_plan(cfg, inputs['edge_user'],
                                     inputs['edge_movie'])
    nc = build_program(cfg, streams, gidx[0].shape[1], dloc[0].shape[1],
                       cfg.NCORES)
    in_maps = make_in_maps(cfg, gidx, dloc, inputs)
    res = bass_utils.run_bass_kernel_spmd(nc, in_maps,
                                          core_ids=list(range(cfg.NCORES)))
    return assemble(cfg, res.results)



# revision 4
# speedup vs baseline: 1.0083x; 1.0083x over previous
"""GCMC layer on trn2 — v2: ap_gather (GPSIMD free-dim gather) + one-hot PE
segment-sum in transformed (16-lane) message space.

Design (per device, dst-sharded: device c owns dst nodes [c*NSH, (c+1)*NSH)):
  - Transform-first: x_r = (feat*cj) @ W_r  ([N,16] per rating) so each edge
    only moves 16 lanes. Table per (side, g): SBUF [128, 32000] f32 where
    partition 16k+j = lane j of slab (8g+k) (slab=6400 srcs), elem r*6400+s.
  - Edges binned per (d, g, unit k=src slab, dst-tile t, rating r); each
    (t, r) run padded to a 128-multiple of the max count over (device, unit)
    so the SPMD program is uniform. Unit streams are position-aligned: at any
    chunk all 8 units are in the same (t, r) run.
  - nc.gpsimd.ap_gather pulls per-edge x_r lanes from the SBUF table
    (per-unit int16 idx lists; Pool engine, ~6-9ns/idx/unit, 8 units in
    parallel — replaces dma_gather's ~8.6ns/row serial descriptor gen).
  - Per 128-row position: PE transpose -> T [128 e, 128 (k,j)]; DVE builds 8
    one-hots from dloc codes; 8 matmuls accumulate psum_y[dst,16] per run;
    DVE flushes into yacc [128, TP, 80] bf16.
  - Table build: stage fsrc16=(feat*cj) bf16 in DRAM; per slab DMA-transpose
    -> FT [64, 6400]; xT = Wall^T @ FT on PE; DVE copy psum->xTslab f32;
    5 HWDGE DMAs partition-remap xTslab -> table unit block.
  - Transform per (d, tile): relu(msg*ci) on ACT, PE transpose, f32 fc matmul
    + bias, dense store.
Host only bins/sorts indices and packs int16 idx + f32 dloc blocks.
"""
import sys
import numpy as np

sys.path.insert(0, '/opt/trn_rl_repo')

import concourse.bass as bass
import concourse.tile as tile
import concourse.mybir as mybir
from concourse import bacc
from concourse.masks import make_identity

F32 = mybir.dt.float32
BF16 = mybir.dt.bfloat16
I16 = mybir.dt.int16
ALU = mybir.AluOpType
ACTF = mybir.ActivationFunctionType
P = 128


class Cfg:
    def __init__(self, NU=100000, NM=100000, R=5, E=1000000, IN=64, BAS=4,
                 NCORES=8):
        assert NU == NM
        self.NU, self.NM, self.R, self.E, self.IN, self.BAS = NU, NM, R, E, IN, BAS
        self.MPR = 16
        self.MSG = self.MPR * R              # 80
        self.OUT = 64
        self.NCORES = NCORES
        self.NSH = NU // NCORES              # dsts per device
        self.TP = -(-self.NSH // P)          # dst tiles (98)
        self.UN = 8                          # gpsimd units
        self.SLAB = 6400                     # srcs per unit-slab
        self.NG = 2                          # slab groups (16 slabs total)
        self.NELEM = self.R * self.SLAB      # 32000 table elems per partition
        self.NUP = self.SLAB * self.UN * self.NG   # padded src count 102400
        self.NI = 6144                       # ap_gather rows per call


# ----------------------------------------------------------------- host prep

def build_plan(cfg, edge_user, edge_movie):
    """Bin edges per (d, g, unit, tile, rating); pad each (t, r) run to a
    128-multiple of the max count across (device, unit).

    Returns:
      chunks: per (d, g): list of (t, r, first, last) per 128-row chunk
      rowsdg: per (d, g): padded rows per unit
      gidx:  per-device int16 [128, G16] idx blocks (concat over d, g)
      dl8:   per-device f32  [128, CC*8] dloc blocks (-1 = padding)
    """
    NC, UN, TP, R = cfg.NCORES, cfg.UN, cfg.TP, cfg.R
    NSH, SLAB = cfg.NSH, cfg.SLAB
    eu = np.asarray(edge_user)
    em = np.asarray(edge_movie)

    chunks_all = []
    rowsdg = []
    gparts = [[] for _ in range(NC)]
    dparts = [[] for _ in range(NC)]
    for d in range(2):
        src_all, dst_all = (eu, em) if d == 0 else (em, eu)
        src = src_all.reshape(-1).astype(np.int64)
        dst = dst_all.reshape(-1).astype(np.int64)
        rr = np.repeat(np.arange(R, dtype=np.int64), cfg.E)
        c = dst // NSH
        ld = dst % NSH
        t = ld // P
        dl = (ld % P).astype(np.float32)
        slab = src // SLAB
        g = slab // UN
        k = slab % UN
        s = src % SLAB
        idx = (rr * SLAB + s).astype(np.int16)
        for gv in range(2):
            m = g == gv
            key = ((c[m] * UN + k[m]) * TP + t[m]) * R + rr[m]
            cnt = np.bincount(key, minlength=NC * UN * TP * R)
            cnt = cnt.reshape(NC, UN, TP, R)
            nch = -(-cnt.max(axis=(0, 1)) // P)          # [TP, R] chunks
            L = nch * P
            Lf = L.reshape(-1)
            base = np.concatenate([[0], np.cumsum(Lf)[:-1]]).reshape(TP, R)
            rows = int(Lf.sum())
            rowsdg.append(rows)
            # rank within (c,k,t,r)
            order = np.argsort(key, kind='stable')
            ko = key[order]
            bnd = np.flatnonzero(np.diff(ko, prepend=-1))
            rank = np.arange(ko.size) - np.repeat(
                bnd, np.diff(np.append(bnd, ko.size)))
            inv = np.empty_like(order)
            inv[order] = np.arange(order.size)
            rank = rank[inv]
            pos = base[t[m], rr[m]] + rank
            gs = np.zeros((NC, UN, rows), np.int16)
            dv = np.full((NC, UN, rows), -1.0, np.float32)
            gs[c[m], k[m], pos] = idx[m]
            dv[c[m], k[m], pos] = dl[m]
            for cc in range(NC):
                gb = np.zeros((P, rows // 16), np.int16)
                db = np.zeros((P, (rows // P) * UN), np.float32)
                dbv = db.reshape(P, rows // P, UN)
                for kk in range(UN):
                    gb[16 * kk:16 * kk + 16] = \
                        gs[cc, kk].reshape(-1, 16).T
                    dbv[:, :, kk] = dv[cc, kk].reshape(-1, P).T
                gparts[cc].append(gb)
                dparts[cc].append(db)
            ch = []
            for tt in range(TP):
                for r in range(R):
                    n = int(nch[tt, r])
                    for j in range(n):
                        ch.append((tt, r, j == 0, j == n - 1))
            chunks_all.append(ch)
    gidx = [np.concatenate(gp, axis=1) for gp in gparts]
    dl8 = [np.concatenate(dp, axis=1) for dp in dparts]
    return chunks_all, rowsdg, gidx, dl8


# ------------------------------------------------------------- numpy model

def model(cfg, chunks_all, rowsdg, gidx, dl8, inputs):
    """Numpy mirror of the device program (f32, no bf16 rounding)."""
    import ml_dtypes
    BF = ml_dtypes.bfloat16
    R, NSH, TP, SLAB, UN = cfg.R, cfg.NSH, cfg.TP, cfg.SLAB, cfg.UN
    W = np.einsum('rb,bio->rio', np.asarray(inputs['att']),
                  np.asarray(inputs['basis'])).astype(np.float32)
    fc_w = np.asarray(inputs['fc_w'])
    fc_b = np.asarray(inputs['fc_b'])
    xfull = np.zeros((2, cfg.NUP, cfg.MSG), np.float32)
    for side, (f, cj) in enumerate((('ufeat', 'cj_user'), ('ifeat', 'cj_movie'))):
        fc = (np.asarray(inputs[f]) * np.asarray(inputs[cj])).astype(BF)
        for r in range(R):
            xfull[side, :cfg.NU, 16 * r:16 * r + 16] = \
                fc.astype(np.float32) @ W[r].astype(BF).astype(np.float32)
    ci = [np.asarray(inputs['ci_movie']), np.asarray(inputs['ci_user'])]

    u_out = np.zeros((cfg.NU, cfg.OUT), np.float32)
    m_out = np.zeros((cfg.NM, cfg.OUT), np.float32)
    for c in range(cfg.NCORES):
        g16o = 0
        cco = 0
        for d in range(2):
            yacc = np.zeros((TP * P, cfg.MSG), np.float32)
            for g in range(2):
                ch = chunks_all[d * 2 + g]
                rows = rowsdg[d * 2 + g]
                gb = gidx[c][:, g16o:g16o + rows // 16]
                db = dl8[c][:, cco:cco + (rows // P) * UN].reshape(
                    P, rows // P, UN)
                g16o += rows // 16
                cco += (rows // P) * UN
                trow = np.repeat([t for (t, r, _, _) in ch], P)
                rrow = np.repeat([r for (t, r, _, _) in ch], P)
                for k in range(UN):
                    idxs = gb[16 * k:16 * k + 16].T.reshape(-1)  # [rows]
                    dls = db[:, :, k].T.reshape(-1)
                    msk = dls >= 0
                    base = (8 * g + k) * SLAB
                    elem = idxs.astype(np.int64)
                    s = elem % SLAB
                    rv = elem // SLAB
                    val = np.zeros((rows, 16), np.float32)
                    sel = xfull[d, base + s]                      # [rows, 80]
                    val = sel[np.arange(rows)[:, None],
                              (rv * 16)[:, None] + np.arange(16)[None, :]]
                    tgt = trow * P + dls.astype(np.int64)
                    col = rrow * 16
                    np.add.at(yacc, (tgt[msk][:, None],
                                     (col[msk][:, None] + np.arange(16))),
                              val[msk])
            cish = np.zeros((TP * P, 1), np.float32)
            cish[:NSH] = ci[d][c * NSH:(c + 1) * NSH]
            z = np.maximum(yacc * cish, 0.0) @ fc_w.T + fc_b
            if d == 0:
                m_out[c * NSH:(c + 1) * NSH] = z[:NSH]
            else:
                u_out[c * NSH:(c + 1) * NSH] = z[:NSH]
    return u_out, m_out


# ---------------------------------------------------------- device program

def build_program(cfg, chunks_all, rowsdg, g16cols, cccols, num_devices):
    nc = bacc.Bacc("TRN2", target_bir_lowering=False, debug=False,
                   num_devices=num_devices)
    NU, IN, R, BAS = cfg.NU, cfg.IN, cfg.R, cfg.BAS
    TP, UN, SLAB, NELEM, MSG, NI = (cfg.TP, cfg.UN, cfg.SLAB, cfg.NELEM,
                                    cfg.MSG, cfg.NI)

    ufeat = nc.dram_tensor("ufeat", (NU, IN), F32, kind="ExternalInput")
    ifeat = nc.dram_tensor("ifeat", (NU, IN), F32, kind="ExternalInput")
    cj_u = nc.dram_tensor("cj_u", (NU, 1), F32, kind="ExternalInput")
    cj_m = nc.dram_tensor("cj_m", (NU, 1), F32, kind="ExternalInput")
    ci_sh = nc.dram_tensor("ci_sh", (2 * TP * P, 1), F32, kind="ExternalInput")
    attT = nc.dram_tensor("attT", (BAS, R), F32, kind="ExternalInput")
    basis2 = nc.dram_tensor("basis2", (BAS, IN * 16), F32, kind="ExternalInput")
    fc_w = nc.dram_tensor("fc_w", (64, MSG), F32, kind="ExternalInput")
    fc_b = nc.dram_tensor("fc_b", (1, 64), F32, kind="ExternalInput")
    iota_d = nc.dram_tensor("iota128", (P, P), BF16, kind="ExternalInput")
    gidx = nc.dram_tensor("gidx", (P, g16cols), I16, kind="ExternalInput")
    dl8 = nc.dram_tensor("dl8", (P, cccols), BF16, kind="ExternalInput")

    m_out = nc.dram_tensor("m_out", (TP * P, 64), F32, kind="ExternalOutput")
    u_out = nc.dram_tensor("u_out", (TP * P, 64), F32, kind="ExternalOutput")

    wscr = nc.dram_tensor("wscr", (R, IN * 16), F32, kind="Internal")
    fsrc16 = nc.dram_tensor("fsrc16", (2 * cfg.NUP, IN), BF16, kind="Internal")

    with tile.TileContext(nc) as tc:
        with tc.tile_pool(name="const", bufs=1) as pool:
            # ---------------- constants ----------------
            pp_ctx = tc.tile_pool(name="cpsum", bufs=2, space="PSUM")
            pp = pp_ctx.__enter__()
            ident = pool.tile([P, P], F32)
            make_identity(nc, ident[:])
            ident16 = pool.tile([P, P], BF16)
            make_identity(nc, ident16[:])

            with tc.tile_pool(name="w0", bufs=1) as wp:
                at = wp.tile([BAS, R], F32)
                bs = wp.tile([BAS, IN * 16], F32)
                nc.sync.dma_start(out=at[:], in_=attT.ap()[:])
                nc.sync.dma_start(out=bs[:], in_=basis2.ap()[:])
                w5 = wp.tile([R, IN * 16], F32)
                half = IN * 16 // 2
                for h in range(2):
                    ps = pp.tile([R, half], F32, space="PSUM", tag="w5ps")
                    nc.tensor.matmul(out=ps[:], lhsT=at[:],
                                     rhs=bs[:, h * half:(h + 1) * half],
                                     start=True, stop=True)
                    nc.scalar.copy(out=w5[:, h * half:(h + 1) * half], in_=ps[:])
                nc.sync.dma_start(out=wscr.ap()[:], in_=w5[:])
            w64 = pool.tile([IN, R, 16], F32)
            nc.sync.dma_start(
                out=w64[:], in_=wscr.ap()[:].rearrange("r (k o) -> k r o", k=IN))
            wallb = pool.tile([IN, R * 16], BF16)
            nc.scalar.copy(out=wallb[:],
                           in_=w64[:].rearrange("k r o -> k (r o)"))

            fcw = pool.tile([64, MSG], F32)
            nc.sync.dma_start(out=fcw[:], in_=fc_w.ap()[:])
            psT = pp.tile([MSG, 64], F32, space="PSUM", tag="fcT")
            nc.tensor.transpose(out=psT[:], in_=fcw[:], identity=ident[:64, :64])
            fcwT = pool.tile([MSG, 64], F32)
            nc.scalar.copy(out=fcwT[:], in_=psT[:])
            fcb = pool.tile([P, 64], F32)
            nc.sync.dma_start(out=fcb[:], in_=fc_b.ap()[:].to_broadcast((P, 64)))

            cisb = pool.tile([P, 2 * TP], F32)
            nc.sync.dma_start(
                out=cisb[:],
                in_=ci_sh.ap()[:].rearrange("(t p) o -> p (t o)", p=P))
            iota = pool.tile([P, 1, P], BF16)
            nc.sync.dma_start(out=iota[:, 0, :], in_=iota_d.ap()[:])
            pp_ctx.__exit__(None, None, None)

            # ---------------- stage fsrc16 = (feat*cj) bf16 ----------------
            with tc.tile_pool(name="p1", bufs=3) as p1:
                GT = 8
                for side, (feat, cj) in enumerate(((ufeat, cj_u), (ifeat, cj_m))):
                    starts = list(range(0, NU - GT * P + 1, GT * P))
                    if NU % (GT * P):
                        starts.append(NU - GT * P)
                    for g0 in starts:
                        ft = p1.tile([P, GT, IN], F32, tag="ft")
                        cjt = p1.tile([P, GT, 1], F32, tag="cj")
                        nc.sync.dma_start(
                            out=ft[:], in_=feat.ap()[g0:g0 + GT * P].rearrange(
                                "(p a) d -> p a d", p=P))
                        nc.sync.dma_start(
                            out=cjt[:], in_=cj.ap()[g0:g0 + GT * P].rearrange(
                                "(p a) d -> p a d", p=P))
                        sc = p1.tile([P, GT, IN], BF16, tag="sc")
                        nc.vector.tensor_tensor(
                            out=sc[:], in0=ft[:],
                            in1=cjt[:].to_broadcast((P, GT, IN)),
                            op=ALU.mult)
                        ofs = side * cfg.NUP + g0
                        nc.sync.dma_start(
                            out=fsrc16.ap()[ofs:ofs + GT * P]
                                .rearrange("(p a) d -> p a d", p=P),
                            in_=sc[:])

            # ---------------- main: per direction ----------------
            g16ofs = 0
            ccofs = 0
            with tc.tile_pool(name="tblp", bufs=1) as tblp, \
                 tc.tile_pool(name="yaccp", bufs=1) as yp:
                tbl = tblp.tile([P, NELEM, 1], F32)
                for d in range(2):
                    yacc = yp.tile([P, TP, MSG], BF16, tag="yacc")
                    nc.vector.memset(yacc[:], 0.0)
                    for g in range(2):
                        # ---- table build for (side=d, group g) ----
                        with tc.tile_pool(name="tb", bufs=3) as tb, \
                             tc.tile_pool(name="tbx", bufs=1) as tbx, \
                             tc.tile_pool(name="tbps", bufs=3,
                                          space="PSUM") as tbps:
                            for k in range(UN):
                                base = d * cfg.NUP + (UN * g + k) * SLAB
                                for hh in range(2):
                                    half = SLAB // 2
                                    xts = tbx.tile([MSG, half], F32,
                                                   tag="xts")
                                    for c0 in range(0, half, 512):
                                        w = min(512, half - c0)
                                        fc4 = tb.tile([P, 4, IN], BF16,
                                                      tag="fc4")
                                        nc.sync.dma_start(
                                            out=fc4[:, :w // P, :],
                                            in_=fsrc16.ap()[
                                                base + hh * half + c0:
                                                base + hh * half + c0 + w]
                                                .rearrange("(a p) d -> p a d",
                                                           p=P))
                                        ft4 = tb.tile([IN, 4, P], BF16,
                                                      tag="ft4")
                                        for j in range(w // P):
                                            fps = tbps.tile(
                                                [IN, P], BF16, space="PSUM",
                                                tag="fps")
                                            nc.tensor.transpose(
                                                out=fps[:],
                                                in_=fc4[:, j, :],
                                                identity=ident16[:])
                                            nc.vector.tensor_copy(
                                                out=ft4[:, j, :], in_=fps[:])
                                        xps = tbps.tile([MSG, 512], F32,
                                                        space="PSUM", tag="xps")
                                        nc.tensor.matmul(
                                            out=xps[:, :w],
                                            lhsT=wallb[:],
                                            rhs=ft4[:].rearrange(
                                                "f a p -> f (a p)")[:, :w],
                                            start=True, stop=True)
                                        nc.vector.tensor_copy(
                                            out=xts[:, c0:c0 + w],
                                            in_=xps[:, :w])
                                    for r in range(R):
                                        nc.sync.dma_start(
                                            out=tbl[16 * k:16 * k + 16,
                                                    r * SLAB + hh * (SLAB // 2):
                                                    r * SLAB + (hh + 1) * (SLAB // 2),
                                                    0],
                                            in_=xts[16 * r:16 * r + 16, :])
                        # ---- gather + segment-sum for (d, g) ----
                        chunksl = chunks_all[d * 2 + g]
                        rows = rowsdg[d * 2 + g]
                        with tc.tile_pool(name="io", bufs=2) as iop, \
                             tc.tile_pool(name="go", bufs=1) as gop, \
                             tc.tile_pool(name="gb", bufs=1) as gbp, \
                             tc.tile_pool(name="oh", bufs=3) as ohp, \
                             tc.tile_pool(name="ts", bufs=3) as tsp, \
                             tc.tile_pool(name="mps", bufs=4,
                                          space="PSUM") as mps:
                            pos = 0
                            psy = None
                            for a0 in range(0, rows, NI):
                                ni = min(NI, rows - a0)
                                gi = iop.tile([P, NI // 16], I16, tag="gi")
                                nc.sync.dma_start(
                                    out=gi[:, :ni // 16],
                                    in_=gidx.ap()[:, g16ofs + a0 // 16:
                                                  g16ofs + (a0 + ni) // 16])
                                dlt = iop.tile([P, NI // P, UN, 1], BF16,
                                               tag="dlt")
                                nc.sync.dma_start(
                                    out=dlt[:, :ni // P, :, :],
                                    in_=dl8.ap()[:, ccofs + (a0 // P) * UN:
                                                 ccofs + ((a0 + ni) // P) * UN]
                                        .rearrange("p (c k one) -> p c k one",
                                                   k=UN, one=1))
                                go = gop.tile([P, NI, 1], F32, tag="go")
                                nc.gpsimd.ap_gather(
                                    go[:, :ni, :], tbl[:], gi[:, :ni // 16],
                                    channels=P, num_elems=NELEM, d=1,
                                    num_idxs=ni)
                                gb = gbp.tile([P, NI], BF16, tag="gbc")
                                nc.vector.tensor_copy(out=gb[:, :ni],
                                                      in_=go[:, :ni, 0])
                                for local in range(ni // P):
                                    t, r, first, last = chunksl[pos]
                                    pos += 1
                                    tps = mps.tile([P, P], BF16, space="PSUM",
                                                   tag="tps")
                                    nc.tensor.transpose(
                                        out=tps[:],
                                        in_=gb[:, local * P:(local + 1) * P],
                                        identity=ident16[:])
                                    tsb = tsp.tile([P, P], BF16, tag="tsb")
                                    nc.vector.tensor_copy(out=tsb[:], in_=tps[:])
                                    oh = ohp.tile([P, UN, P], BF16, tag="oh")
                                    nc.vector.tensor_tensor(
                                        out=oh[:],
                                        in0=dlt[:, local, :, :].to_broadcast(
                                            (P, UN, P)),
                                        in1=iota[:, 0:1, :].to_broadcast(
                                            (P, UN, P)),
                                        op=ALU.is_equal)
                                    if first:
                                        psy = mps.tile([P, 16], F32,
                                                       space="PSUM", tag="psy")
                                    for k in range(UN):
                                        nc.tensor.matmul(
                                            out=psy[:],
                                            lhsT=oh[:, k, :],
                                            rhs=tsb[:, 16 * k:16 * k + 16],
                                            start=(first and k == 0),
                                            stop=(last and k == UN - 1))
                                    if last:
                                        ys = yacc[:, t, r * 16:(r + 1) * 16]
                                        nc.vector.tensor_tensor(
                                            out=ys, in0=ys, in1=psy[:],
                                            op=ALU.add)
                        g16ofs += rows // 16
                        ccofs += (rows // P) * UN
                    # ---------------- transform ----------------
                    with tc.tile_pool(name="p3", bufs=3) as p3, \
                         tc.tile_pool(name="p3ps", bufs=2, space="PSUM") as p3p:
                        for t in range(TP):
                            msg = p3.tile([P, MSG], F32, tag="msg")
                            nc.scalar.activation(
                                out=msg[:], in_=yacc[:, t, :],
                                func=ACTF.Relu,
                                scale=cisb[:, d * TP + t: d * TP + t + 1])
                            psmT = p3p.tile([MSG, P], F32, space="PSUM",
                                            tag="psmT")
                            nc.tensor.transpose(out=psmT[:], in_=msg[:],
                                                identity=ident[:])
                            msgT = p3.tile([MSG, P], F32, tag="msgT")
                            nc.vector.tensor_copy(out=msgT[:], in_=psmT[:])
                            fcp = p3p.tile([P, 64], F32, space="PSUM",
                                           tag="fcp")
                            nc.tensor.matmul(
                                out=fcp[:], lhsT=msgT[:], rhs=fcwT[:],
                                start=True, stop=True)
                            osb = p3.tile([P, 64], F32, tag="osb")
                            nc.vector.tensor_tensor(out=osb[:], in0=fcp[:],
                                                    in1=fcb[:], op=ALU.add)
                            dst = m_out if d == 0 else u_out
                            nc.sync.dma_start(
                                out=dst.ap()[t * P:(t + 1) * P], in_=osb[:])
    nc.compile()
    return nc


# ----------------------------------------------------------------- kernel

def make_in_maps(cfg, gidx, dl8, inputs):
    import ml_dtypes
    ins = {k: np.asarray(v) for k, v in inputs.items()}
    iota = np.tile(np.arange(P, dtype=ml_dtypes.bfloat16), (P, 1))
    dl8 = [d.astype(ml_dtypes.bfloat16) for d in dl8]
    base = dict(
        ufeat=ins['ufeat'], ifeat=ins['ifeat'],
        cj_u=ins['cj_user'], cj_m=ins['cj_movie'],
        attT=np.ascontiguousarray(ins['att'].T),
        basis2=ins['basis'].reshape(cfg.BAS, cfg.IN * 16).copy(),
        fc_w=ins['fc_w'], fc_b=ins['fc_b'].reshape(1, 64).copy(),
        iota128=iota,
    )
    in_maps = []
    for c in range(cfg.NCORES):
        ci = np.zeros((2 * cfg.TP * P, 1), np.float32)
        ci[:cfg.NSH] = ins['ci_movie'][c * cfg.NSH:(c + 1) * cfg.NSH]
        ci[cfg.TP * P:cfg.TP * P + cfg.NSH] = \
            ins['ci_user'][c * cfg.NSH:(c + 1) * cfg.NSH]
        in_maps.append({**base, 'ci_sh': ci, 'gidx': gidx[c], 'dl8': dl8[c]})
    return in_maps


def assemble(cfg, results):
    u = np.concatenate([results[c]['u_out'][:cfg.NSH]
                        for c in range(cfg.NCORES)])
    m = np.concatenate([results[c]['m_out'][:cfg.NSH]
                        for c in range(cfg.NCORES)])
    return u, m


def kernel(**inputs):
    from concourse import bass_utils
    cfg = Cfg()
    chunks_all, rowsdg, gidx, dl8 = build_plan(cfg, inputs['edge_user'],
                                               inputs['edge_movie'])
    nc = build_program(cfg, chunks_all, rowsdg, gidx[0].shape[1],
                       dl8[0].shape[1], cfg.NCORES)
    in_maps = make_in_maps(cfg, gidx, dl8, inputs)
    res = bass_utils.run_bass_kernel_spmd(nc, in_maps,
                                          core_ids=list(range(cfg.NCORES)))
    return assemble(cfg, res.results)
